# revision 7
# baseline (speedup 1.0000x reference)
"""Trainium2 Bass kernel for nn_CrossAttention1d (B=8, C=768, N=256, H=12, D=64).

Math (per batch b), algebraically equal to the reference but avoiding the
[3072, 3072] attention matrix via associativity:

    cp_full = (scale * W_proj) @ cross_b + scale * b_proj    [C, N]
    CP = cp_full.reshape(D, H*N)   (pure reshape)
    Xc = cross_b.reshape(D, H*N)   (pure reshape)
    K  = CP @ Xc^T                                           [D, D] (pre-scaled)
    X  = x_ori_b.reshape(D, H*N)
    OT = K^T @ X                                             [D, H*N]
    out2T[h*64+d, n] = OT[d, n*12+h]                         [C, N]
    yT = W_dep @ out2T + b_dep                               [C, N]
    out_b = x_ori_b + yT

Sharding: data-parallel over batch, one batch per NeuronCore (8 cores).

Schedule (per core), designed around the DMA stream being the binding
resource (~3.9MB bf16 at ~360GB/s ~= 11us):
  - inputs arrive in consumer order on the SP HWDGE queue: biases, cross,
    wp chunk 0, crossT (host-uploaded, replaces PE transposes), wp chunks
    1-2, xq, xr, then wd in 6 chunks.
  - proj accumulates per wp chunk as it lands (4 psum groups, t-outer),
    bias via rank-1 ones-matmul, evictions (bf16) split DVE/Act.
  - K accumulated over 24 bf16 [128]x[64]x[64] matmuls (scale pre-folded
    into wp on host); K duplicated to partitions [64:128] for OT.
  - OT parity-split (even m -> partitions [0:64], odd -> [64:128]) so the
    deproj rhs is a single stride-6 AP at full K=128.
  - deproj: per-oi accumulation groups opened early with bias (ones-matmul)
    + residual (identity-matmul of xr), then 6 matmuls per wd chunk as it
    streams in; 3 [128,512] psum banks evicted straight to out, with the
    out DMAs on the Act HWDGE queue so they never block input DMA issue.
  - PE warmup matmuls in the prologue climb the p-state ramp during the
    initial DMA wait.
  - loop build: body = 2 unrolled iterations on disjoint tile sets (manual
    double buffering), PSUM pools alternate left/right allocation sides.
"""

import numpy as np

import concourse.bacc as bacc
import concourse.mybir as mybir
import concourse.tile as tile
from concourse.bass_utils import run_bass_kernel_spmd
from concourse.masks import make_identity

B, C, N = 8, 768, 256
H, D = 12, 64
M = H * N  # 3072
SCALE = float(D) ** -0.5
N_CORES = 8
F32 = mybir.dt.float32
BF16 = mybir.dt.bfloat16

WARMUP_MM = 8

_built_nc = None


def _declare(nc):
    xq = nc.dram_tensor("xq", [128, M // 2], BF16, kind="ExternalInput")
    xr = nc.dram_tensor("xr", [128, 6 * N], BF16, kind="ExternalInput")
    cr = nc.dram_tensor("cr", [128, 6 * N], BF16, kind="ExternalInput")
    ct = nc.dram_tensor("ct", [128, 2 * C], BF16, kind="ExternalInput")
    wp = nc.dram_tensor("wp", [128, 6 * C], BF16, kind="ExternalInput")
    wd = nc.dram_tensor("wd", [128, 6 * C], BF16, kind="ExternalInput")
    bpd = nc.dram_tensor("bpd", [1, 2 * C], BF16, kind="ExternalInput")
    out = nc.dram_tensor("out", [128, 6 * N], BF16, kind="ExternalOutput")
    return dict(xq=xq, xr=xr, cr=cr, ct=ct, wp=wp, wd=wd, bpd=bpd, out=out)


def make_const(tc, nc, sb, dram):
    """Constants + prologue: identity/ones, bias DMA, act-table preload,
    PE p-state warmup."""
    Copy = mybir.ActivationFunctionType.Copy
    ident = sb.tile([128, 128], BF16, name="ident")
    ones = sb.tile([1, 256], BF16, name="ones")
    bpd_sb = sb.tile([1, 2 * C], BF16, name="bpd_sb")
    scratch = sb.tile([1, 256], BF16, name="scratch")
    nc.sync.dma_start(bpd_sb[:], dram["bpd"].ap())
    nc.gpsimd.memset(ones[:], 1.0)
    make_identity(nc, ident[:])
    # act-table preload so the first real Act eviction doesn't pay 1.3us
    nc.scalar.activation(scratch[:], ones[:], Copy)
    if WARMUP_MM:
        with tc.tile_pool(name="pwu", bufs=1, space="PSUM") as pwu:
            wps = pwu.tile([128, 256], F32)
            for i in range(WARMUP_MM):
                nc.tensor.matmul(
                    wps[:], ones[0:1, 0:128], ones[0:1, 0:256],
                    start=(i == 0), stop=(i == WARMUP_MM - 1),
                )
    return dict(ident=ident, ones=ones, bpd=bpd_sb)


def alloc_tiles(sb, tag):
    t = {}
    t["cr"] = sb.tile([128, 6 * N], BF16, name=f"cr{tag}")
    t["ct"] = sb.tile([128, 2 * C], BF16, name=f"ct{tag}")
    t["wp"] = sb.tile([128, 6 * C], BF16, name=f"wp{tag}")
    t["xq"] = sb.tile([128, M // 2], BF16, name=f"xq{tag}")
    t["xr"] = sb.tile([128, 6 * N], BF16, name=f"xr{tag}")
    t["wd"] = sb.tile([128, 6 * C], BF16, name=f"wd{tag}")
    t["cpT"] = sb.tile([128, 2 * C], BF16, name=f"cpT{tag}")
    t["k"] = sb.tile([128, 64], BF16, name=f"k{tag}")
    t["ot2"] = sb.tile([128, M // 2], BF16, name=f"ot2{tag}")
    t["out"] = sb.tile([128, 6 * N], BF16, name=f"out{tag}")
    return t


def emit(tc, nc, dram, const, t, parity):
    """One iteration. `t`: this iteration's tile set. `parity` picks the
    PSUM allocation side so unrolled iterations don't collide on banks."""
    add = mybir.AluOpType.add
    Copy = mybir.ActivationFunctionType.Copy
    pside = ("left", "right")[parity]

    ident, ones, bpd = const["ident"], const["ones"], const["bpd"]

    # ---- input DMAs, consumer order, all on the SP queue ----------------
    nc.sync.dma_start(t["cr"][:], dram["cr"].ap())
    nc.sync.dma_start(t["wp"][:, 0:1536], dram["wp"].ap()[:, 0:1536])
    nc.sync.dma_start(t["ct"][:], dram["ct"].ap())
    nc.sync.dma_start(t["wp"][:, 1536:3072], dram["wp"].ap()[:, 1536:3072])
    nc.sync.dma_start(t["wp"][:, 3072:4608], dram["wp"].ap()[:, 3072:4608])
    nc.sync.dma_start(t["xq"][:], dram["xq"].ap())
    nc.sync.dma_start(t["xr"][:], dram["xr"].ap())
    for j in range(6):
        nc.sync.dma_start(
            t["wd"][:, j * C:(j + 1) * C], dram["wd"].ap()[:, j * C:(j + 1) * C]
        )

    # ---- proj: cpT[n, o] = sum_c cross[c, n] wpT[c, o] + bias -----------
    with tc.tile_pool(name=f"ppj{parity}", bufs=4, space="PSUM", side=pside) as ppj:
        pps = [ppj.tile([128, 384], F32, name=f"pps{parity}_{g}", tag="pps")
               for g in range(4)]
        for tt in range(6):
            for ni in range(2):
                for oj in range(2):
                    nc.tensor.matmul(
                        pps[2 * ni + oj][:],
                        t["cr"][:, tt * N + ni * 128: tt * N + ni * 128 + 128],
                        t["wp"][:, tt * C + oj * 384: tt * C + oj * 384 + 384],
                        start=(tt == 0),
                        stop=False,
                    )
        for ni in range(2):
            for oj in range(2):
                g = 2 * ni + oj
                nc.tensor.matmul(
                    pps[g][:],
                    ones[0:1, 0:128],
                    bpd[0:1, oj * 384:(oj + 1) * 384],
                    start=False,
                    stop=True,
                )
                dst = t["cpT"][:, ni * C + oj * 384: ni * C + oj * 384 + 384]
                if g % 2 == 0:
                    nc.vector.tensor_copy(dst, pps[g][:])
                else:
                    nc.scalar.activation(dst, pps[g][:], Copy)

    # ---- K and OT --------------------------------------------------------
    with (
        tc.tile_pool(name=f"pk{parity}", bufs=1, space="PSUM", side=pside) as pk,
        tc.tile_pool(name=f"pot{parity}", bufs=3, space="PSUM", side=pside) as pot,
    ):
        kps = pk.tile([64, 64], F32, name=f"kps{parity}")
        cpT_v = t["cpT"][:].rearrange("p (c d h) -> p c h d", c=2, h=H)
        crT_v = t["ct"][:].rearrange("p (c d h) -> p c h d", c=2, h=H)
        first = True
        for h in range(H):
            for ni in range(2):
                nc.tensor.matmul(
                    kps[:],
                    cpT_v[:, ni, h],
                    crT_v[:, ni, h],
                    start=first,
                    stop=(h == H - 1 and ni == 1),
                )
                first = False
        nc.scalar.activation(t["k"][0:64, :], kps[:], Copy)
        nc.scalar.activation(t["k"][64:128, :], kps[:], Copy)

        # OT parity-split: even m -> partitions [0:64], odd m -> [64:128]
        x_v = t["xq"][:].rearrange("p (tt par) -> p par tt", par=2)
        for j in range(6):
            half, sub = j // 3, j % 3
            hb = half * 64
            po = pot.tile([128, 256], F32, name=f"po{parity}_{j}", tag="po")
            nc.tensor.matmul(
                po[0:64, :],
                t["k"][hb:hb + 64, :],
                x_v[hb:hb + 64, 0, sub * 256:(sub + 1) * 256],
                start=True, stop=True,
            )
            nc.tensor.matmul(
                po[64:128, :],
                t["k"][hb:hb + 64, :],
                x_v[hb:hb + 64, 1, sub * 256:(sub + 1) * 256],
                start=True, stop=True,
            )
            dst = t["ot2"][:, j * 256:(j + 1) * 256]
            if j % 2 == 0:
                nc.vector.tensor_copy(dst, po[:])
            else:
                nc.scalar.activation(dst, po[:], Copy)

    # ---- deproj + bias + residual, streaming with the wd chunks ---------
    with tc.tile_pool(name=f"py{parity}", bufs=6, space="PSUM", side=pside) as py:
        yts = [py.tile([128, 256], F32, name=f"yt{parity}_{p}", tag="yt")
               for p in range(6)]

        def yv(oi):
            return yts[oi][:]

        ot2_v = t["ot2"][:].rearrange("p (tt six) -> p six tt", six=6)
        for oi in range(6):
            # open the accumulation group with bias, add the residual
            nc.tensor.matmul(
                yv(oi),
                bpd[0:1, C + oi * 128: C + (oi + 1) * 128],
                ones[0:1, 0:256],
                start=True, stop=False,
            )
            nc.tensor.matmul(
                yv(oi),
                ident[:, 0:128],
                t["xr"][:, oi * N:(oi + 1) * N],
                start=False, stop=False,
            )
        for j in range(6):
            for oi in range(6):
                nc.tensor.matmul(
                    yv(oi),
                    t["wd"][:, j * C + oi * 128: j * C + oi * 128 + 128],
                    ot2_v[:, j],
                    start=False,
                    stop=(j == 5),
                )
        for oi in range(6):
            dst = t["out"][:, oi * N:(oi + 1) * N]
            if oi % 2 == 0:
                nc.vector.tensor_copy(dst, yts[oi][:])
            else:
                nc.scalar.activation(dst, yts[oi][:], Copy)
                # out DMA on the Act HWDGE queue: never blocks input DMA issue
                nc.scalar.dma_start(
                    dram["out"].ap()[:, (oi - 1) * N:(oi + 1) * N],
                    t["out"][:, (oi - 1) * N:(oi + 1) * N],
                )


def build(reps=None):
    nc = bacc.Bacc("TRN2", target_bir_lowering=False, debug=False)
    dram = _declare(nc)
    with tile.TileContext(nc) as tc:
        with tc.tile_pool(name="sb", bufs=1) as sb:
            const = make_const(tc, nc, sb, dram)
            if reps is None or reps == 1:
                tA = alloc_tiles(sb, "a")
                if reps is None:
                    emit(tc, nc, dram, const, tA, 0)
                else:
                    with tc.For_i(0, 1, 1, hint_engines=(mybir.EngineType.PE,)):
                        emit(tc, nc, dram, const, tA, 0)
            else:
                assert reps % 2 == 0
                tA = alloc_tiles(sb, "a")
                tB = alloc_tiles(sb, "b")
                with tc.For_i(0, reps // 2, 1, hint_engines=(mybir.EngineType.PE,)):
                    emit(tc, nc, dram, const, tA, 0)
                    emit(tc, nc, dram, const, tB, 1)
    nc.compile()
    return nc


def build_loop(reps):
    return build(reps=reps)


def make_in_maps(x_ori, cross, W_proj, b_proj, W_dep, b_dep):
    import ml_dtypes

    wdt = ml_dtypes.bfloat16
    x_ori = np.asarray(x_ori, np.float32)
    cross = np.asarray(cross, np.float32)

    def w_perm(w):  # [C, C] W^T -> [128, 4608] SBUF layout
        return np.ascontiguousarray(
            w.T.reshape(2, 3, 128, C).transpose(2, 0, 1, 3).reshape(128, 6 * C)
            .astype(wdt)
        )

    def tn_perm(a):  # [C, N] -> [128, (t n)]
        return np.ascontiguousarray(
            a.reshape(6, 128, N).transpose(1, 0, 2).reshape(128, 6 * N).astype(wdt)
        )

    def ct_perm(a):  # [C, N] -> crossT [128, (ni c)] with n = ni*128 + p
        return np.ascontiguousarray(
            a.T.reshape(2, 128, C).transpose(1, 0, 2).reshape(128, 2 * C).astype(wdt)
        )

    def xq_perm(a):  # [C, N] -> [128, 1536], p = half*64+d, f = m - half*1536
        return np.ascontiguousarray(
            a.reshape(D, 2, M // 2).transpose(1, 0, 2).reshape(128, M // 2)
            .astype(wdt)
        )

    wp = w_perm(np.asarray(W_proj, np.float32) * SCALE)
    wd = w_perm(np.asarray(W_dep, np.float32))
    bpd = np.concatenate(
        [np.asarray(b_proj, np.float32) * SCALE, np.asarray(b_dep, np.float32)]
    ).reshape(1, 2 * C).astype(wdt)
    bpd = np.ascontiguousarray(bpd)
    return [
        {
            "xq": xq_perm(x_ori[b]),
            "xr": tn_perm(x_ori[b]),
            "cr": tn_perm(cross[b]),
            "ct": ct_perm(cross[b]),
            "wp": wp,
            "wd": wd,
            "bpd": bpd,
        }
        for b in range(B)
    ]


def unpermute_out(o):  # [128, (t n)] -> [C, N]
    return np.asarray(o, np.float32).reshape(128, 6, N).transpose(1, 0, 2).reshape(C, N)


def kernel(**inputs):
    global _built_nc
    if _built_nc is None:
        _built_nc = build()
    nc = _built_nc
    in_maps = make_in_maps(
        inputs["x_ori"], inputs["cross"], inputs["W_proj"],
        inputs["b_proj"], inputs["W_dep"], inputs["b_dep"],
    )
    res = run_bass_kernel_spmd(nc, in_maps, list(range(N_CORES)))
    out = np.stack([unpermute_out(res.results[c]["out"]) for c in range(N_CORES)])
    return out.astype(np.float32)
